# revision 8
# baseline (speedup 1.0000x reference)
"""Delay-and-sum beamformer on 8 TRN2 NeuronCores.

Problem: x[16, 100000, 128] f32 -> out[b, t] = mean_s x[b, t + d_s, s],
d_s = round(s * sin(30deg) / 2) in [0, 32] (zero-padded past t = T-1).

Sharding: pure data parallel over batch (2 batches per core).

Per-core layout ("stripe" scheme): for each batch, partition p owns time
rows [784*p, 784*(p+1)) of a zero-padded T_pad = 100384 signal.  Each
partition's rows live contiguously in its SBUF free dim as [row, sensor]
(row pitch 128 f32), loaded in 14 chunks of 56 rows through a 6-slot ring
with a 32-row shadow after slot 5.  Chunks are processed in descending
time order so a chunk's +32-row halo is always already resident; when the
ring wraps (slot 5), the halo is SBUF-copied from slot 0 into the shadow.

The delayed sensor sum exploits the delay structure: d groups sensors as
{0,1,2} (d=0), {4k-1..4k+2} (d=k, k=1..31), {127} (d=32).  In the
flattened per-partition layout the taps for output row tau sit at
  tau*128 + {0,1,2},  tau*128 + 131 + 132*k + j (k=0..30, j=0..3),
  tau*128 + 4223,
so the 128-sensor shifted sum is two strided tensor_reduce ops + two adds
on the vector engine; the final 1/S scale runs on the scalar engine.

DMA engine plan: chunk loads stream on the Sync HWDGE ring (qSP) only, in
strict FIFO order, so the oldest chunk always completes first.  Output
stores ride the second HWDGE ring (qAct via the scalar engine) so their
semaphore waits never head-of-line-block loads at the sync sequencer.
Halo copies run as Copy activations on the (otherwise idle) scalar
engine's compute side: SBUF engine lanes are physically separate from
the DMA/AXI ports, so the copies cost zero SDMA-queue time.  The deep
6-chunk ring keeps ~5 chunks of descriptors queued ahead of the
computes, absorbing transient SDMA-engine slowdowns.
"""

import numpy as np

B, T, S = 16, 100000, 128
NCORES = 8
BC = B // NCORES          # batches per core
LS = 784                  # stripe rows per partition (128*784 = 100352 >= T)
TP = 128 * LS             # padded output rows per batch
HALO = 32                 # max delay
TPAD = TP + HALO          # padded input rows per batch
LC = 56                   # chunk rows
NCH = LS // LC            # 14 chunks per batch
NSLOT = 6                 # ring slots
RING_ROWS = NSLOT * LC + HALO  # 6 slots + shadow
SCALE = 1.0 / S
HLS = LS // 2             # half-batch output rows per store

_cache = {}


def _build():
    import concourse.bass as bass
    import concourse.tile as tile
    from concourse import bacc, mybir

    f32 = mybir.dt.float32
    nc = bacc.Bacc("TRN2", target_bir_lowering=False, debug=False, num_devices=1)
    x = nc.dram_tensor("x", [BC * TPAD * S], f32, kind="ExternalInput")
    y = nc.dram_tensor("y", [BC * TP], f32, kind="ExternalOutput")

    def dram_ap(base_elem, rows):
        # [128 partitions (stripe-major), rows*S contiguous elems each]
        return bass.AP(x.ap().tensor, base_elem, [[LS * S, 128], [1, rows * S]])

    def sub_ap(t, off, dims):
        # custom AP into a tile: keep its partition dim, replace free dims
        return bass.AP(t.tensor, t.offset + off, [list(t.ap[0])] + dims)

    with tile.TileContext(nc) as tc:
        from contextlib import ExitStack

        with ExitStack() as ctx:
            ring_pool = ctx.enter_context(tc.tile_pool(name="ring", bufs=1))
            g_pool = ctx.enter_context(tc.tile_pool(name="g", bufs=2))
            o_pool = ctx.enter_context(tc.tile_pool(name="o", bufs=2))

            ring = ring_pool.tile([128, RING_ROWS * S], f32)

            def compute(c, slot, out_sb, t0=0, t1=LC):
                # produce out rows [c*LC+t0, c*LC+t1) from ring rows
                # [slot*LC+t0, slot*LC+t1+32)
                n = t1 - t0
                base = (slot * LC + t0) * S
                g1 = g_pool.tile([128, n], f32, tag="g1")
                g2 = g_pool.tile([128, n], f32, tag="g2")
                # groups k=1..31 (sensors 3..126): rows tau+1..tau+31
                nc.vector.reduce_sum(
                    g1[:],
                    sub_ap(ring, base + 131, [[S, n], [132, 31], [1, 4]]),
                    axis=mybir.AxisListType.XY,
                )
                # group d=0 (sensors 0..2) at row tau
                nc.vector.reduce_sum(
                    g2[:],
                    sub_ap(ring, base, [[S, n], [1, 3]]),
                    axis=mybir.AxisListType.X,
                )
                nc.vector.tensor_add(g1[:], g1[:], g2[:])
                # sensor 127 (d=32) at row tau+32, col 127
                nc.vector.tensor_add(
                    g1[:], g1[:], sub_ap(ring, base + 4223, [[S, n]])
                )
                # final 1/S scale on the scalar engine (keeps DVE lean)
                nc.scalar.activation(
                    out_sb[:, c * LC + t0 : c * LC + t1],
                    g1[:],
                    mybir.ActivationFunctionType.Copy,
                    scale=SCALE,
                )

            def store(out_sb, b, lo, hi):
                # store out rows [lo, hi) of batch b on the qAct ring
                nc.scalar.dma_start(
                    bass.AP(y.ap().tensor, b * TP + lo, [[LS, 128], [1, hi - lo]]),
                    out_sb[:, lo:hi],
                )

            # Chunks are processed in DESCENDING order: compute c reads its
            # own chunk plus the first 32 rows of chunk c+1, which is then
            # always already resident.  Slot map slot(c) = (c + off) % 6
            # keeps compute windows contiguous.  Slot 5 is followed by the
            # 32-row shadow; on slot-5 chunks (other than a batch's first)
            # the shadow is SBUF-copied from the resident slot-0 head.
            #   b0: off=4 -> slot(13)=5: the first (fat, 88-row) load also
            #       covers the stripe tail, so no tail DMA.
            #   b1: off=1 -> slot(13)=2: first loads land on slots whose
            #       last b0 readers finish ~4+ computes before b0 ends, so
            #       the batch handoff never stalls the sync sequencer on
            #       b0's final computes.  The stripe tail is loaded into
            #       slot 3's head by a separate 32-row DMA.
            offsets = [4, 1]

            for b in range(BC):
                xb = b * TPAD * S
                off = offsets[b]
                out_sb = o_pool.tile([128, LS], f32, tag="out_sb")
                s13 = (NCH - 1 + off) % NSLOT
                if s13 != NSLOT - 1:
                    # stripe-tail halo for chunk 13's compute: rows
                    # [LS, LS+32) -> the head of the slot after its own.
                    nc.sync.dma_start(
                        sub_ap(ring, (s13 + 1) * LC * S, [[1, HALO * S]]),
                        dram_ap(xb + LS * S, HALO),
                    )
                last = b == BC - 1
                c = NCH - 1
                while c >= 0:
                    slot = (c + off) % NSLOT
                    first_fat = c == NCH - 1 and slot == NSLOT - 1
                    if first_fat or (last and c <= 1):
                        # split into descending sub-units: at the kernel
                        # start this lets the first compute begin sooner;
                        # at the kernel end it shrinks the DVE work left
                        # after the last HBM byte lands.
                        fat = HALO if first_fat else 0
                        units = 4 if last and c == 0 else 2
                        step = LC // units
                        for u in range(units - 1, -1, -1):
                            t0, t1 = u * step, (u + 1) * step
                            rows = t1 - t0 + (fat if u == units - 1 else 0)
                            nc.sync.dma_start(
                                sub_ap(ring, (slot * LC + t0) * S,
                                       [[1, rows * S]]),
                                dram_ap(xb + (c * LC + t0) * S, rows),
                            )
                            compute(c, slot, out_sb, t0, t1)
                        if last and c == 1:
                            # rows [2*LC, HLS) all computed by now
                            store(out_sb, b, 2 * LC, HLS)
                        if last and c == 0:
                            store(out_sb, b, 0, 2 * LC)
                        c -= 1
                        continue
                    # pair chunks (c, c-1) into one 112-row load when they
                    # sit in physically adjacent ring slots and c-1 needs
                    # no special handling: bigger descriptors stream
                    # closer to SDMA line rate and halve the sem traffic.
                    pair = (
                        slot != 0
                        and c >= 1
                        and not (last and c - 1 <= 1)
                    )
                    lo_c = c - 1 if pair else c
                    lo_slot = slot - 1 if pair else slot
                    nc.sync.dma_start(
                        sub_ap(ring, lo_slot * LC * S,
                               [[1, (c - lo_c + 1) * LC * S]]),
                        dram_ap(xb + lo_c * LC * S, (c - lo_c + 1) * LC),
                    )
                    if slot == NSLOT - 1 and c < NCH - 1:
                        # ring wrap: chunk c+1's head (rows [0,32) of slot
                        # 0) is resident; copy it into the shadow on the
                        # scalar engine's compute side (engine-lane SBUF
                        # ports: zero SDMA/AXI cost).
                        nc.scalar.activation(
                            sub_ap(ring, NSLOT * LC * S, [[1, HALO * S]]),
                            sub_ap(ring, 0, [[1, HALO * S]]),
                            mybir.ActivationFunctionType.Copy,
                        )
                    compute(c, slot, out_sb)
                    if pair:
                        compute(c - 1, slot - 1, out_sb)
                    if c == NCH // 2 or (pair and c - 1 == NCH // 2):
                        # upper half of the batch output is complete
                        store(out_sb, b, HLS, LS)
                    c = lo_c - 1
                if not last:
                    store(out_sb, b, 0, HLS)

    nc.compile()
    return nc


def _get_nc():
    if "nc" not in _cache:
        _cache["nc"] = _build()
    return _cache["nc"]


def kernel(microphone_array: np.ndarray) -> np.ndarray:
    from concourse.bass_utils import run_bass_kernel_spmd

    x = np.asarray(microphone_array, dtype=np.float32)
    assert x.shape == (B, T, S)
    nc = _get_nc()

    in_maps = []
    for c in range(NCORES):
        shard = np.zeros((BC, TPAD, S), dtype=np.float32)
        shard[:, :T] = x[c * BC : (c + 1) * BC]
        in_maps.append({"x": shard.reshape(-1)})

    res = _cache["res"] = run_bass_kernel_spmd(
        nc, in_maps, core_ids=list(range(NCORES)), trace=_cache.get("trace", False)
    )

    out = np.empty((B, T), dtype=np.float32)
    for c in range(NCORES):
        out[c * BC : (c + 1) * BC] = res.results[c]["y"].reshape(BC, TP)[:, :T]
    return out


# revision 9
# speedup vs baseline: 1.0356x; 1.0356x over previous
"""Delay-and-sum beamformer on 8 TRN2 NeuronCores.

Problem: x[16, 100000, 128] f32 -> out[b, t] = mean_s x[b, t + d_s, s],
d_s = round(s * sin(30deg) / 2) in [0, 32] (zero-padded past t = T-1).

Sharding: pure data parallel over batch (2 batches per core).

Per-core layout ("stripe" scheme): for each batch, partition p owns time
rows [784*p, 784*(p+1)) of a zero-padded T_pad = 100384 signal.  Each
partition's rows live contiguously in its SBUF free dim as [row, sensor]
(row pitch 128 f32), loaded in 14 chunks of 56 rows through a 6-slot ring
with a 32-row shadow after slot 5.  Chunks are processed in descending
time order so a chunk's +32-row halo is always already resident; when the
ring wraps (slot 5), the halo is SBUF-copied from slot 0 into the shadow.

The delayed sensor sum exploits the delay structure: d groups sensors as
{0,1,2} (d=0), {4k-1..4k+2} (d=k, k=1..31), {127} (d=32).  In the
flattened per-partition layout the taps for output row tau sit at
  tau*128 + {0,1,2},  tau*128 + 131 + 132*k + j (k=0..30, j=0..3),
  tau*128 + 4223,
so the 128-sensor shifted sum is two strided tensor_reduce ops + two adds
on the vector engine; the final 1/S scale runs on the scalar engine.

DMA engine plan: chunk loads stream on the Sync HWDGE ring (qSP) only, in
strict FIFO order, so the oldest chunk always completes first.  Output
stores ride the second HWDGE ring (qAct via the scalar engine) so their
semaphore waits never head-of-line-block loads at the sync sequencer.
Halo copies run as Copy activations on the (otherwise idle) scalar
engine's compute side: SBUF engine lanes are physically separate from
the DMA/AXI ports, so the copies cost zero SDMA-queue time.  The deep
6-chunk ring keeps ~5 chunks of descriptors queued ahead of the
computes, absorbing transient SDMA-engine slowdowns.
"""

import numpy as np

B, T, S = 16, 100000, 128
NCORES = 8
BC = B // NCORES          # batches per core
LS = 784                  # stripe rows per partition (128*784 = 100352 >= T)
TP = 128 * LS             # padded output rows per batch
HALO = 32                 # max delay
TPAD = TP + HALO          # padded input rows per batch
LC = 56                   # chunk rows
NCH = LS // LC            # 14 chunks per batch
NSLOT = 6                 # ring slots
RING_ROWS = NSLOT * LC + HALO  # 6 slots + shadow
SCALE = 1.0 / S
HLS = LS // 2             # half-batch output rows per store

_cache = {}


def _build():
    import concourse.bass as bass
    import concourse.tile as tile
    from concourse import bacc, mybir

    f32 = mybir.dt.float32
    nc = bacc.Bacc("TRN2", target_bir_lowering=False, debug=False, num_devices=1)
    x = nc.dram_tensor("x", [BC * TPAD * S], f32, kind="ExternalInput")
    y = nc.dram_tensor("y", [BC * TP], f32, kind="ExternalOutput")

    def dram_ap(base_elem, rows):
        # [128 partitions (stripe-major), rows*S contiguous elems each]
        return bass.AP(x.ap().tensor, base_elem, [[LS * S, 128], [1, rows * S]])

    def sub_ap(t, off, dims):
        # custom AP into a tile: keep its partition dim, replace free dims
        return bass.AP(t.tensor, t.offset + off, [list(t.ap[0])] + dims)

    with tile.TileContext(nc) as tc:
        from contextlib import ExitStack

        with ExitStack() as ctx:
            ring_pool = ctx.enter_context(tc.tile_pool(name="ring", bufs=1))
            g_pool = ctx.enter_context(tc.tile_pool(name="g", bufs=2))
            o_pool = ctx.enter_context(tc.tile_pool(name="o", bufs=2))

            ring = ring_pool.tile([128, RING_ROWS * S], f32)

            def compute(c, slot, out_sb, t0=0, t1=LC):
                # produce out rows [c*LC+t0, c*LC+t1) from ring rows
                # [slot*LC+t0, slot*LC+t1+32)
                n = t1 - t0
                base = (slot * LC + t0) * S
                g1 = g_pool.tile([128, n], f32, tag="g1")
                g2 = g_pool.tile([128, n], f32, tag="g2")
                # groups k=1..31 (sensors 3..126): rows tau+1..tau+31
                nc.vector.reduce_sum(
                    g1[:],
                    sub_ap(ring, base + 131, [[S, n], [132, 31], [1, 4]]),
                    axis=mybir.AxisListType.XY,
                )
                # group d=0 (sensors 0..2) at row tau
                nc.vector.reduce_sum(
                    g2[:],
                    sub_ap(ring, base, [[S, n], [1, 3]]),
                    axis=mybir.AxisListType.X,
                )
                nc.vector.tensor_add(g1[:], g1[:], g2[:])
                # sensor 127 (d=32) at row tau+32, col 127
                nc.vector.tensor_add(
                    g1[:], g1[:], sub_ap(ring, base + 4223, [[S, n]])
                )
                # final 1/S scale on the scalar engine (keeps DVE lean)
                nc.scalar.activation(
                    out_sb[:, c * LC + t0 : c * LC + t1],
                    g1[:],
                    mybir.ActivationFunctionType.Copy,
                    scale=SCALE,
                )

            def store(out_sb, b, lo, hi):
                # store out rows [lo, hi) of batch b on the qAct ring
                nc.scalar.dma_start(
                    bass.AP(y.ap().tensor, b * TP + lo, [[LS, 128], [1, hi - lo]]),
                    out_sb[:, lo:hi],
                )

            # Chunks are processed in DESCENDING order: compute c reads its
            # own chunk plus the first 32 rows of chunk c+1, which is then
            # always already resident.  Slot map slot(c) = (c + off) % 6
            # keeps compute windows contiguous.  Slot 5 is followed by the
            # 32-row shadow; on slot-5 chunks (other than a batch's first)
            # the shadow is SBUF-copied from the resident slot-0 head.
            #   b0: off=4 -> slot(13)=5: the first (fat, 88-row) load also
            #       covers the stripe tail, so no tail DMA.
            #   b1: off=1 -> slot(13)=2: first loads land on slots whose
            #       last b0 readers finish ~4+ computes before b0 ends, so
            #       the batch handoff never stalls the sync sequencer on
            #       b0's final computes.  The stripe tail is loaded into
            #       slot 3's head by a separate 32-row DMA.
            offsets = [4, 1]

            for b in range(BC):
                xb = b * TPAD * S
                off = offsets[b]
                out_sb = o_pool.tile([128, LS], f32, tag="out_sb")
                s13 = (NCH - 1 + off) % NSLOT
                if s13 != NSLOT - 1:
                    # stripe-tail halo for chunk 13's compute: rows
                    # [LS, LS+32) -> the head of the slot after its own.
                    nc.sync.dma_start(
                        sub_ap(ring, (s13 + 1) * LC * S, [[1, HALO * S]]),
                        dram_ap(xb + LS * S, HALO),
                    )
                last = b == BC - 1
                c = NCH - 1
                while c >= 0:
                    slot = (c + off) % NSLOT
                    first_fat = c == NCH - 1 and slot == NSLOT - 1
                    if first_fat or (last and c <= 1):
                        # split into descending sub-units: at the kernel
                        # start this lets the first compute begin sooner;
                        # at the kernel end it shrinks the DVE work left
                        # after the last HBM byte lands.
                        fat = HALO if first_fat else 0
                        units = 4 if last and c == 0 else 2
                        step = LC // units
                        for u in range(units - 1, -1, -1):
                            t0, t1 = u * step, (u + 1) * step
                            rows = t1 - t0 + (fat if u == units - 1 else 0)
                            nc.sync.dma_start(
                                sub_ap(ring, (slot * LC + t0) * S,
                                       [[1, rows * S]]),
                                dram_ap(xb + (c * LC + t0) * S, rows),
                            )
                            compute(c, slot, out_sb, t0, t1)
                        if last and c == 1:
                            # rows [2*LC, HLS) all computed by now
                            store(out_sb, b, 2 * LC, HLS)
                        if last and c == 0:
                            store(out_sb, b, 0, 2 * LC)
                        c -= 1
                        continue
                    nc.sync.dma_start(
                        sub_ap(ring, slot * LC * S, [[1, LC * S]]),
                        dram_ap(xb + c * LC * S, LC),
                    )
                    if slot == NSLOT - 1 and c < NCH - 1:
                        # ring wrap: chunk c+1's head (rows [0,32) of slot
                        # 0) is resident; copy it into the shadow on the
                        # scalar engine's compute side (engine-lane SBUF
                        # ports: zero SDMA/AXI cost).
                        nc.scalar.activation(
                            sub_ap(ring, NSLOT * LC * S, [[1, HALO * S]]),
                            sub_ap(ring, 0, [[1, HALO * S]]),
                            mybir.ActivationFunctionType.Copy,
                        )
                    compute(c, slot, out_sb)
                    if c == NCH // 2:
                        # upper half of the batch output is complete
                        store(out_sb, b, HLS, LS)
                    c -= 1
                if not last:
                    store(out_sb, b, 0, HLS)

    nc.compile()
    return nc


def _get_nc():
    if "nc" not in _cache:
        _cache["nc"] = _build()
    return _cache["nc"]


def kernel(microphone_array: np.ndarray) -> np.ndarray:
    from concourse.bass_utils import run_bass_kernel_spmd

    x = np.asarray(microphone_array, dtype=np.float32)
    assert x.shape == (B, T, S)
    nc = _get_nc()

    in_maps = []
    for c in range(NCORES):
        shard = np.zeros((BC, TPAD, S), dtype=np.float32)
        shard[:, :T] = x[c * BC : (c + 1) * BC]
        in_maps.append({"x": shard.reshape(-1)})

    res = _cache["res"] = run_bass_kernel_spmd(
        nc, in_maps, core_ids=list(range(NCORES)), trace=_cache.get("trace", False)
    )

    out = np.empty((B, T), dtype=np.float32)
    for c in range(NCORES):
        out[c * BC : (c + 1) * BC] = res.results[c]["y"].reshape(BC, TP)[:, :T]
    return out
